# revision 1
# baseline (speedup 1.0000x reference)
import numpy as np

import concourse.bass as bass
import concourse.bacc as bacc
import concourse.tile as tile
from concourse import mybir
from concourse.bass_utils import run_bass_kernel_spmd

B, T, F, U, NCLS = 512, 512, 128, 64, 10
NCORES = 8
BC = B // NCORES          # 64 batch rows per core
WS = 8                    # timesteps per PSUM window
NW = T // WS              # 64 windows
TCH = 64                  # timesteps per DMA chunk
NCHUNK = T // TCH
WIN_PER_CHUNK = TCH // WS

f32 = mybir.dt.float32
AF = mybir.ActivationFunctionType
OP = mybir.AluOpType

TRACE = False
LAST_RESULTS = None


def build_nc(nzrec: bool, nzb0h: bool) -> bass.Bass:
    nc = bacc.Bacc(None, target_bir_lowering=False)

    x = nc.dram_tensor("x", [BC, T, F], f32, kind="ExternalInput")
    Wzr = nc.dram_tensor("Wzr", [F, 2 * U], f32, kind="ExternalInput")
    Wh = nc.dram_tensor("Wh", [F, U], f32, kind="ExternalInput")
    Uzr = nc.dram_tensor("Uzr", [U, 2 * U], f32, kind="ExternalInput")
    Uh = nc.dram_tensor("Uh", [U, U], f32, kind="ExternalInput")
    W1 = nc.dram_tensor("W1", [U, U], f32, kind="ExternalInput")
    W2 = nc.dram_tensor("W2", [U, NCLS], f32, kind="ExternalInput")
    sclzr = nc.dram_tensor("sclzr", [2 * U, 1], f32, kind="ExternalInput")
    bzr = nc.dram_tensor("bzr", [2 * U, 1], f32, kind="ExternalInput")
    b1h = nc.dram_tensor("b1h", [U, 1], f32, kind="ExternalInput")
    b0h = nc.dram_tensor("b0h", [U, 1], f32, kind="ExternalInput")
    b1v = nc.dram_tensor("b1v", [U, 1], f32, kind="ExternalInput")
    b2v = nc.dram_tensor("b2v", [NCLS, 1], f32, kind="ExternalInput")
    ident = nc.dram_tensor("ident", [U, U], f32, kind="ExternalInput")
    out = nc.dram_tensor("out", [BC, NCLS], f32, kind="ExternalOutput")

    with tile.TileContext(nc) as tc:
        with (
            tc.tile_pool(name="const", bufs=1) as cpool,
            tc.tile_pool(name="xchunk", bufs=2) as xpool,
            tc.tile_pool(name="xtw", bufs=2) as xtwpool,
            tc.tile_pool(name="hbuf", bufs=1) as hpool,
            tc.tile_pool(name="spool", bufs=3) as spool,
            tc.tile_pool(name="dpool", bufs=3) as dpool,
        ):
            # ---- constants to SBUF ----
            wzr_sb = cpool.tile([F, 2 * U], f32)
            nc.sync.dma_start(wzr_sb, Wzr[:, :])
            wh_sb = cpool.tile([F, U], f32)
            nc.sync.dma_start(wh_sb, Wh[:, :])
            uzr_sb = cpool.tile([U, 2 * U], f32)
            nc.sync.dma_start(uzr_sb, Uzr[:, :])
            uh_sb = cpool.tile([U, U], f32)
            nc.sync.dma_start(uh_sb, Uh[:, :])
            w1_sb = cpool.tile([U, U], f32)
            nc.sync.dma_start(w1_sb, W1[:, :])
            w2_sb = cpool.tile([U, NCLS], f32)
            nc.sync.dma_start(w2_sb, W2[:, :])
            sclzr_sb = cpool.tile([2 * U, 1], f32)
            nc.sync.dma_start(sclzr_sb, sclzr[:, :])
            bzr_sb = cpool.tile([2 * U, 1], f32)
            nc.sync.dma_start(bzr_sb, bzr[:, :])
            b1h_sb = cpool.tile([U, 1], f32)
            nc.sync.dma_start(b1h_sb, b1h[:, :])
            b0h_sb = cpool.tile([U, 1], f32)
            nc.sync.dma_start(b0h_sb, b0h[:, :])
            b1v_sb = cpool.tile([U, 1], f32)
            nc.sync.dma_start(b1v_sb, b1v[:, :])
            b2v_sb = cpool.tile([NCLS, 1], f32)
            nc.sync.dma_start(b2v_sb, b2v[:, :])
            ident_sb = cpool.tile([U, U], f32)
            nc.sync.dma_start(ident_sb, ident[:, :])

            # PE matmuls may carry only ONE sync wait (LDW struct limit).
            # Route every const through a DVE copy so PE instrs only ever
            # wait on compute semaphores, never raw DMA semaphores.
            def dve_copy(src, shape):
                dst = cpool.tile(shape, f32, name=src.tensor.name + "_c")
                nc.vector.tensor_copy(dst, src)
                return dst

            wzr_c = dve_copy(wzr_sb, [F, 2 * U])
            wh_c = dve_copy(wh_sb, [F, U])
            uzr_c = dve_copy(uzr_sb, [U, 2 * U])
            uh_c = dve_copy(uh_sb, [U, U])
            w1_c = dve_copy(w1_sb, [U, U])
            w2_c = dve_copy(w2_sb, [U, NCLS])
            ident_c = dve_copy(ident_sb, [U, U])
            sclzr_c = dve_copy(sclzr_sb, [2 * U, 1])
            bzr_c = dve_copy(bzr_sb, [2 * U, 1])
            b1h_c = dve_copy(b1h_sb, [U, 1])
            b0h_c = dve_copy(b0h_sb, [U, 1])
            b1v_c = dve_copy(b1v_sb, [U, 1])
            b2v_c = dve_copy(b2v_sb, [NCLS, 1])

            # ---- recurrent state (ping-pong) ----
            hA = hpool.tile([U, BC], f32, name="hA")
            hB = hpool.tile([U, BC], f32, name="hB")
            nc.vector.memset(hA, 0.0)

            xs_tiles = {}

            def emit_dma(c):
                xsb = xpool.tile([BC, TCH, F], f32, name="xsb")
                nc.sync.dma_start(xsb, x[:, c * TCH:(c + 1) * TCH, :])
                xs_tiles[c] = xsb

            with (
                tc.tile_pool(name="pzr", bufs=2, space="PSUM") as pZR,
                tc.tile_pool(name="pxh", bufs=2, space="PSUM") as pXH,
                tc.tile_pool(name="ptr", bufs=2, space="PSUM") as pTR,
                tc.tile_pool(name="prh", bufs=2, space="PSUM") as pRH,
            ):
                def make_bulk(w):
                    c = (w * WS) // TCH
                    xsb = xs_tiles[c]
                    pstr = pTR.tile([F, WS * BC], f32, name="pstr")
                    xtw = xtwpool.tile([F, WS * BC], f32, name="xtw")
                    pszr = pZR.tile([2 * U, WS * BC], f32, name="pszr")
                    psxh = pXH.tile([U, WS * BC], f32, name="psxh")
                    tasks = []
                    for j in range(WS):
                        lt = w * WS + j - c * TCH

                        def tr(j=j, lt=lt, xsb=xsb, pstr=pstr):
                            nc.tensor.matmul(
                                pstr[:, j * BC:(j + 1) * BC],
                                xsb[:, lt, :], ident_c,
                                is_transpose=True, skip_group_check=True,
                            )
                        tasks.append(tr)
                    tasks.append(lambda: nc.scalar.copy(xtw, pstr))
                    tasks.append(lambda: nc.tensor.matmul(
                        pszr, wzr_c, xtw, start=True, stop=False,
                        skip_group_check=True))
                    tasks.append(lambda: nc.tensor.matmul(
                        psxh, wh_c, xtw, start=True, stop=True))
                    return (pszr, psxh), tasks

                # absorb the DVE const-copy threshold on PE so the first
                # transposes only carry the DMA wait (LDW allows 1 sem wait)
                dummy = pRH.tile([U, BC], f32, name="rh")
                nc.tensor.matmul(dummy, ident_c, ident_c, start=True, stop=True)

                handles = {}
                emit_dma(0)
                handles[0], t0 = make_bulk(0)
                for t in t0:
                    t()

                for w in range(NW):
                    if w % WIN_PER_CHUNK == 0:
                        c = w // WIN_PER_CHUNK + 1
                        if c < NCHUNK:
                            emit_dma(c)
                    if w + 1 < NW:
                        handles[w + 1], ntasks = make_bulk(w + 1)
                    else:
                        ntasks = []
                    pszr, psxh = handles.pop(w)
                    ti = 0
                    for j in range(WS):
                        t = w * WS + j
                        cur = hA if t % 2 == 0 else hB
                        nxt = hB if t % 2 == 0 else hA
                        sl = slice(j * BC, (j + 1) * BC)
                        nc.tensor.matmul(
                            pszr[:, sl], uzr_c, cur,
                            start=False, stop=True, skip_group_check=True)
                        rh = pRH.tile([U, BC], f32, name="rh")
                        nc.tensor.matmul(rh, uh_c, cur, start=True, stop=True)
                        S = spool.tile([2 * U, BC], f32, name="S")
                        nc.scalar.activation(
                            S, pszr[:, sl], AF.Sigmoid,
                            bias=bzr_c, scale=sclzr_c)
                        p = dpool.tile([U, BC], f32, name="p")
                        if nzrec:
                            nc.vector.scalar_tensor_tensor(
                                p, rh, b1h_c, S[U:2 * U, :],
                                op0=OP.add, op1=OP.mult)
                        else:
                            nc.vector.tensor_mul(p, rh, S[U:2 * U, :])
                        s_ = dpool.tile([U, BC], f32, name="s_")
                        if nzb0h:
                            nc.vector.scalar_tensor_tensor(
                                s_, p, b0h_c, psxh[:, sl],
                                op0=OP.add, op1=OP.add)
                        else:
                            nc.vector.tensor_add(s_, p, psxh[:, sl])
                        g = dpool.tile([U, BC], f32, name="g")
                        nc.vector.scalar_tensor_tensor(
                            g, s_, 0.0, cur, op0=OP.max, op1=OP.subtract)
                        m = dpool.tile([U, BC], f32, name="m")
                        nc.vector.tensor_mul(m, S[:U, :], g)
                        nc.vector.tensor_add(nxt, cur, m)
                        for _ in range(2):
                            if ti < len(ntasks):
                                ntasks[ti]()
                                ti += 1
                    while ti < len(ntasks):
                        ntasks[ti]()
                        ti += 1

            # ---- final MLP + softmax (PSUM banks now free) ----
            with (
                tc.tile_pool(name="pfin", bufs=1, space="PSUM") as pfin,
                tc.tile_pool(name="fpool", bufs=1) as fpool,
            ):
                hF = hA  # 512 steps -> even -> state back in hA
                ps_x = pfin.tile([U, BC], f32)
                nc.tensor.matmul(ps_x, w1_c, hF, start=True, stop=True)
                xT = fpool.tile([U, BC], f32)
                nc.scalar.activation(xT, ps_x, AF.Relu, bias=b1v_c, scale=1.0)
                ps_l = pfin.tile([NCLS, BC], f32)
                nc.tensor.matmul(ps_l, w2_c, xT, start=True, stop=True)
                lg = fpool.tile([NCLS, BC], f32)
                nc.scalar.activation(lg, ps_l, AF.Identity,
                                     bias=b2v_c, scale=1.0)
                ps_t = pfin.tile([BC, NCLS], f32)
                nc.tensor.matmul(ps_t, lg, ident_c[:NCLS, :NCLS],
                                 is_transpose=True, skip_group_check=True)
                lgT = fpool.tile([BC, NCLS], f32)
                nc.scalar.copy(lgT, ps_t)
                mx = fpool.tile([BC, 1], f32)
                nc.vector.tensor_reduce(mx, lgT, axis=mybir.AxisListType.X,
                                        op=OP.max)
                mxn = fpool.tile([BC, 1], f32)
                nc.vector.tensor_scalar_mul(mxn, mx, -1.0)
                ex = fpool.tile([BC, NCLS], f32)
                den = fpool.tile([BC, 1], f32)
                nc.scalar.activation(ex, lgT, AF.Exp, bias=mxn, scale=1.0,
                                     accum_out=den)
                rcp = fpool.tile([BC, 1], f32)
                nc.vector.reciprocal(rcp, den)
                res = fpool.tile([BC, NCLS], f32)
                nc.vector.tensor_scalar_mul(res, ex, rcp)
                nc.sync.dma_start(out[:, :], res)

    nc.finalize()
    return nc


_CACHE = {}


def kernel(**inputs) -> np.ndarray:
    global LAST_RESULTS
    x = np.ascontiguousarray(np.asarray(inputs["inputs"], dtype=np.float32))
    W = np.asarray(inputs["W"], dtype=np.float32)
    Um = np.asarray(inputs["U"], dtype=np.float32)
    b = np.asarray(inputs["b"], dtype=np.float32)
    W1 = np.ascontiguousarray(np.asarray(inputs["W1"], dtype=np.float32))
    b1 = np.asarray(inputs["b1"], dtype=np.float32)
    W2 = np.ascontiguousarray(np.asarray(inputs["W2"], dtype=np.float32))
    b2 = np.asarray(inputs["b2"], dtype=np.float32)

    nzrec = bool(np.any(b[1, 2 * U:]))
    nzb0h = bool(np.any(b[0, 2 * U:]))
    key = (nzrec, nzb0h)
    if key not in _CACHE:
        _CACHE[key] = build_nc(nzrec, nzb0h)
    nc = _CACHE[key]

    bsum = b[0] + b[1]
    bzr = np.concatenate([-bsum[:U], bsum[U:2 * U]]).reshape(2 * U, 1)
    sclzr = np.concatenate([-np.ones(U, np.float32),
                            np.ones(U, np.float32)]).reshape(2 * U, 1)
    common = {
        "Wzr": np.ascontiguousarray(W[:, :2 * U]),
        "Wh": np.ascontiguousarray(W[:, 2 * U:]),
        "Uzr": np.ascontiguousarray(Um[:, :2 * U]),
        "Uh": np.ascontiguousarray(Um[:, 2 * U:]),
        "W1": W1,
        "W2": W2,
        "sclzr": np.ascontiguousarray(sclzr, dtype=np.float32),
        "bzr": np.ascontiguousarray(bzr, dtype=np.float32),
        "b1h": np.ascontiguousarray(b[1, 2 * U:].reshape(U, 1)),
        "b0h": np.ascontiguousarray(b[0, 2 * U:].reshape(U, 1)),
        "b1v": np.ascontiguousarray(b1.reshape(U, 1)),
        "b2v": np.ascontiguousarray(b2.reshape(NCLS, 1)),
        "ident": np.eye(U, dtype=np.float32),
    }
    in_maps = [dict(common, x=np.ascontiguousarray(x[c * BC:(c + 1) * BC]))
               for c in range(NCORES)]
    res = run_bass_kernel_spmd(nc, in_maps, core_ids=list(range(NCORES)),
                               trace=TRACE)
    LAST_RESULTS = res
    return np.concatenate([res.results[c]["out"] for c in range(NCORES)],
                          axis=0).astype(np.float32)



# revision 2
# speedup vs baseline: 1.6484x; 1.6484x over previous
import numpy as np
import ml_dtypes

import concourse.bass as bass
import concourse.bacc as bacc
import concourse.tile as tile
from concourse import mybir
from concourse.bass_utils import run_bass_kernel_spmd

B, T, F, U, NCLS = 512, 512, 128, 64, 10
NCORES = 8
BC = B // NCORES          # 64 batch rows per core
WS = 8                    # timesteps per PSUM window
NW = T // WS              # 64 windows
TCH = 64                  # timesteps per DMA chunk
NCHUNK = T // TCH
WIN_PER_CHUNK = TCH // WS

f32 = mybir.dt.float32
bf16 = mybir.dt.bfloat16
AF = mybir.ActivationFunctionType
OP = mybir.AluOpType
BF = ml_dtypes.bfloat16

TRACE = False
LAST_RESULTS = None


def build_nc(nzrec: bool, nzb0h: bool) -> bass.Bass:
    nc = bacc.Bacc(None, target_bir_lowering=False)

    # x pre-transposed on host: [F, T, BC] bf16
    xT = nc.dram_tensor("xT", [F, T, BC], bf16, kind="ExternalInput")
    Wzr = nc.dram_tensor("Wzr", [F, 2 * U], bf16, kind="ExternalInput")
    Wh = nc.dram_tensor("Wh", [F, U], bf16, kind="ExternalInput")
    Uzr = nc.dram_tensor("Uzr", [U, 2 * U], bf16, kind="ExternalInput")
    Uh = nc.dram_tensor("Uh", [U, U], bf16, kind="ExternalInput")
    sclzr = nc.dram_tensor("sclzr", [2 * U, 1], f32, kind="ExternalInput")
    bzr = nc.dram_tensor("bzr", [2 * U, 1], f32, kind="ExternalInput")
    b1h = nc.dram_tensor("b1h", [U, 1], f32, kind="ExternalInput")
    b0h = nc.dram_tensor("b0h", [U, 1], f32, kind="ExternalInput")
    hout = nc.dram_tensor("hout", [U, BC], bf16, kind="ExternalOutput")

    with tile.TileContext(nc) as tc:
        with (
            tc.tile_pool(name="const", bufs=1) as cpool,
            tc.tile_pool(name="xchunk", bufs=2) as xpool,
            tc.tile_pool(name="hbuf", bufs=1) as hpool,
            tc.tile_pool(name="spool", bufs=3) as spool,
            tc.tile_pool(name="dpool", bufs=3) as dpool,
        ):
            # ---- constants to SBUF ----
            wzr_sb = cpool.tile([F, 2 * U], bf16)
            nc.sync.dma_start(wzr_sb, Wzr[:, :])
            wh_sb = cpool.tile([F, U], bf16)
            nc.sync.dma_start(wh_sb, Wh[:, :])
            uzr_sb = cpool.tile([U, 2 * U], bf16)
            nc.sync.dma_start(uzr_sb, Uzr[:, :])
            uh_sb = cpool.tile([U, U], bf16)
            nc.sync.dma_start(uh_sb, Uh[:, :])
            sclzr_sb = cpool.tile([2 * U, 1], f32)
            nc.sync.dma_start(sclzr_sb, sclzr[:, :])
            bzr_sb = cpool.tile([2 * U, 1], f32)
            nc.sync.dma_start(bzr_sb, bzr[:, :])
            b1h_sb = cpool.tile([U, 1], f32)
            nc.sync.dma_start(b1h_sb, b1h[:, :])
            b0h_sb = cpool.tile([U, 1], f32)
            nc.sync.dma_start(b0h_sb, b0h[:, :])

            # Route consts through DVE copies so PE/Act instrs wait on
            # compute semaphores, not raw DMA semaphores (LDW 1-wait limit).
            def dve_copy(src, shape, dt):
                dst = cpool.tile(shape, dt, name=src.tensor.name + "_c")
                nc.vector.tensor_copy(dst, src)
                return dst

            wzr_c = dve_copy(wzr_sb, [F, 2 * U], bf16)
            wh_c = dve_copy(wh_sb, [F, U], bf16)
            uzr_c = dve_copy(uzr_sb, [U, 2 * U], bf16)
            uh_c = dve_copy(uh_sb, [U, U], bf16)
            sclzr_c = dve_copy(sclzr_sb, [2 * U, 1], f32)
            bzr_c = dve_copy(bzr_sb, [2 * U, 1], f32)
            b1h_c = dve_copy(b1h_sb, [U, 1], f32)
            b0h_c = dve_copy(b0h_sb, [U, 1], f32)

            # ---- recurrent state (ping-pong, bf16) ----
            hA = hpool.tile([U, BC], bf16, name="hA")
            hB = hpool.tile([U, BC], bf16, name="hB")
            nc.vector.memset(hA, 0.0)

            xs_tiles = {}

            def emit_dma(c):
                xsb = xpool.tile([F, TCH, BC], bf16, name="xsb")
                nc.sync.dma_start(xsb, xT[:, c * TCH:(c + 1) * TCH, :])
                xs_tiles[c] = xsb

            with (
                tc.tile_pool(name="pzr", bufs=2, space="PSUM") as pZR,
                tc.tile_pool(name="pxh", bufs=2, space="PSUM") as pXH,
                tc.tile_pool(name="ph", bufs=3, space="PSUM") as pH,
            ):
                # absorb DVE const-copy sem threshold on PE so later PE
                # instrs only carry one data-dependency wait each
                dummy = pH.tile([U, BC], f32, name="rhps")
                nc.tensor.matmul(dummy, uh_c, uh_c, start=True, stop=True)

                pzr_w = {}
                pxh_w = {}

                def emit_bulk(w):
                    c = (w * WS) // TCH
                    xsb = xs_tiles[c]
                    lt = w * WS - c * TCH
                    xsl = xsb[:, lt:lt + WS, :]
                    pzr = pZR.tile([2 * U, WS * BC], f32, name="pzr")
                    pxh = pXH.tile([U, WS * BC], f32, name="pxh")
                    nc.tensor.matmul(pzr, wzr_c, xsl, start=True, stop=False,
                                     skip_group_check=True)
                    nc.tensor.matmul(pxh, wh_c, xsl, start=True, stop=True,
                                     skip_group_check=True)
                    pzr_w[w] = pzr
                    pxh_w[w] = pxh

                emit_dma(0)
                emit_dma(1)
                emit_bulk(0)

                hcur = hA
                hnxt = hB
                D_prev = None
                for t in range(T):
                    w = t // WS
                    sl = slice((t % WS) * BC, (t % WS + 1) * BC)
                    pzr = pzr_w[w]
                    pxh = pxh_w[w]

                    if t >= 1:
                        # critical: close pzr[t] with the D-delta accumulate
                        nc.tensor.matmul(pzr[:, sl], uzr_c, D_prev,
                                         start=False, stop=True,
                                         skip_group_check=True)
                    S = spool.tile([2 * U, BC], f32, name="S")
                    nc.scalar.activation(S, pzr[:, sl], AF.Sigmoid,
                                         bias=bzr_c, scale=sclzr_c)
                    if t >= 1:
                        ph = pH.tile([U, BC], f32, name="rhps")
                        nc.tensor.matmul(ph, uh_c, hcur, start=True, stop=True,
                                         skip_group_check=True)
                        if t + 1 < T:
                            # early accumulate of U^T h_t into pzr[t+1]
                            nxt_w = (t + 1) // WS
                            nsl = slice(((t + 1) % WS) * BC,
                                        ((t + 1) % WS + 1) * BC)
                            nc.tensor.matmul(pzr_w[nxt_w][:, nsl], uzr_c, hcur,
                                             start=False, stop=False,
                                             skip_group_check=True)
                    if t % WS == 0 and w + 1 < NW:
                        emit_bulk(w + 1)
                    if t % TCH == 0 and t // TCH + 2 < NCHUNK:
                        emit_dma(t // TCH + 2)

                    # ---- elementwise chain (DVE) ----
                    if t == 0:
                        C = dpool.tile([U, BC], f32, name="C")
                        nc.vector.tensor_scalar_max(C, pxh[:, sl], 0.0)
                        D = dpool.tile([U, BC], bf16, name="D")
                        nc.vector.tensor_mul(D, S[:U, :], C)
                        nc.vector.tensor_add(hnxt, hcur, D)
                    else:
                        A = dpool.tile([U, BC], f32, name="A")
                        if nzrec:
                            nc.vector.scalar_tensor_tensor(
                                A, ph, b1h_c, S[U:2 * U, :],
                                op0=OP.add, op1=OP.mult)
                        else:
                            nc.vector.tensor_mul(A, S[U:2 * U, :], ph)
                        Bt = dpool.tile([U, BC], f32, name="Bt")
                        if nzb0h:
                            nc.vector.scalar_tensor_tensor(
                                Bt, A, b0h_c, pxh[:, sl],
                                op0=OP.add, op1=OP.add)
                        else:
                            nc.vector.tensor_add(Bt, A, pxh[:, sl])
                        C = dpool.tile([U, BC], f32, name="C")
                        nc.vector.scalar_tensor_tensor(
                            C, Bt, 0.0, hcur, op0=OP.max, op1=OP.subtract)
                        D = dpool.tile([U, BC], bf16, name="D")
                        nc.vector.tensor_mul(D, S[:U, :], C)
                        nc.vector.tensor_add(hnxt, hcur, D)

                    D_prev = D
                    hcur, hnxt = hnxt, hcur

            nc.sync.dma_start(hout[:, :], hcur)

    nc.finalize()
    return nc


_CACHE = {}


def kernel(**inputs) -> np.ndarray:
    global LAST_RESULTS
    x = np.asarray(inputs["inputs"], dtype=np.float32)
    W = np.asarray(inputs["W"], dtype=np.float32)
    Um = np.asarray(inputs["U"], dtype=np.float32)
    b = np.asarray(inputs["b"], dtype=np.float32)
    W1 = np.asarray(inputs["W1"], dtype=np.float32)
    b1 = np.asarray(inputs["b1"], dtype=np.float32)
    W2 = np.asarray(inputs["W2"], dtype=np.float32)
    b2 = np.asarray(inputs["b2"], dtype=np.float32)

    nzrec = bool(np.any(b[1, 2 * U:]))
    nzb0h = bool(np.any(b[0, 2 * U:]))
    key = (nzrec, nzb0h)
    if key not in _CACHE:
        _CACHE[key] = build_nc(nzrec, nzb0h)
    nc = _CACHE[key]

    bsum = b[0] + b[1]
    bzr = np.concatenate([-bsum[:U], bsum[U:2 * U]]).reshape(2 * U, 1)
    sclzr = np.concatenate([-np.ones(U, np.float32),
                            np.ones(U, np.float32)]).reshape(2 * U, 1)
    common = {
        "Wzr": np.ascontiguousarray(W[:, :2 * U]).astype(BF),
        "Wh": np.ascontiguousarray(W[:, 2 * U:]).astype(BF),
        "Uzr": np.ascontiguousarray(Um[:, :2 * U]).astype(BF),
        "Uh": np.ascontiguousarray(Um[:, 2 * U:]).astype(BF),
        "sclzr": np.ascontiguousarray(sclzr, dtype=np.float32),
        "bzr": np.ascontiguousarray(bzr, dtype=np.float32),
        "b1h": np.ascontiguousarray(b[1, 2 * U:].reshape(U, 1)),
        "b0h": np.ascontiguousarray(b[0, 2 * U:].reshape(U, 1)),
    }
    in_maps = []
    for c in range(NCORES):
        xc = np.ascontiguousarray(
            x[c * BC:(c + 1) * BC].transpose(2, 1, 0)).astype(BF)
        in_maps.append(dict(common, xT=xc))
    res = run_bass_kernel_spmd(nc, in_maps, core_ids=list(range(NCORES)),
                               trace=TRACE)
    LAST_RESULTS = res

    # MLP head + softmax on host (fp32)
    h_full = np.concatenate(
        [np.asarray(res.results[c]["hout"]).astype(np.float32).T
         for c in range(NCORES)], axis=0)            # [B, U]
    x1 = np.maximum(h_full @ W1 + b1, 0.0)
    lg = x1 @ W2 + b2
    e = np.exp(lg - lg.max(axis=-1, keepdims=True))
    return (e / e.sum(axis=-1, keepdims=True)).astype(np.float32)


# revision 4
# speedup vs baseline: 1.7295x; 1.0493x over previous
import numpy as np
import ml_dtypes

import concourse.bass as bass
import concourse.bacc as bacc
import concourse.tile as tile
from concourse import mybir
from concourse.bass_utils import run_bass_kernel_spmd

B, T, F, U, NCLS = 512, 512, 128, 64, 10
NCORES = 8
BC = B // NCORES          # 64 batch rows per core
WS = 8                    # timesteps per PSUM window
NW = T // WS              # 64 windows
TCH = 64                  # timesteps per DMA chunk
NCHUNK = T // TCH

f32 = mybir.dt.float32
bf16 = mybir.dt.bfloat16
AF = mybir.ActivationFunctionType
OP = mybir.AluOpType
BF = ml_dtypes.bfloat16

TRACE = False
LAST_RESULTS = None


def build_nc(nzrec: bool, nzb0h: bool, nzzr: bool) -> bass.Bass:
    nc = bacc.Bacc(None, target_bir_lowering=False)

    # x pre-transposed on host: [F, T, BC] bf16.  z-columns of Wzr/Uzr are
    # host-negated so the sigmoid needs no scale vector: S[0:U] = sigmoid(-az)
    # = 1-z, S[U:2U] = sigmoid(ar) = r.
    xT = nc.dram_tensor("xT", [F, T, BC], bf16, kind="ExternalInput")
    Wzr = nc.dram_tensor("Wzr", [F, 2 * U], bf16, kind="ExternalInput")
    Wh = nc.dram_tensor("Wh", [F, U], bf16, kind="ExternalInput")
    Uzr = nc.dram_tensor("Uzr", [U, 2 * U], bf16, kind="ExternalInput")
    Uh = nc.dram_tensor("Uh", [U, U], bf16, kind="ExternalInput")
    bzr = nc.dram_tensor("bzr", [2 * U, 1], f32, kind="ExternalInput")
    b1h = nc.dram_tensor("b1h", [U, 1], f32, kind="ExternalInput")
    b0h = nc.dram_tensor("b0h", [U, 1], f32, kind="ExternalInput")
    hout = nc.dram_tensor("hout", [U, BC], bf16, kind="ExternalOutput")

    with tile.TileContext(nc) as tc:
        with (
            tc.tile_pool(name="const", bufs=1) as cpool,
            tc.tile_pool(name="xchunk", bufs=2) as xpool,
            tc.tile_pool(name="hbuf", bufs=1) as hpool,
            tc.tile_pool(name="spool", bufs=3) as spool,
            tc.tile_pool(name="dpool", bufs=3) as dpool,
            tc.tile_pool(name="ddpool", bufs=2) as ddpool,
            tc.tile_pool(name="xhpool", bufs=3) as xhpool,
        ):
            # ---- constants to SBUF ----
            wzr_sb = cpool.tile([F, 2 * U], bf16)
            nc.sync.dma_start(wzr_sb, Wzr[:, :])
            wh_sb = cpool.tile([F, U], bf16)
            nc.sync.dma_start(wh_sb, Wh[:, :])
            uzr_sb = cpool.tile([U, 2 * U], bf16)
            nc.sync.dma_start(uzr_sb, Uzr[:, :])
            uh_sb = cpool.tile([U, U], bf16)
            nc.sync.dma_start(uh_sb, Uh[:, :])
            bzr_sb = cpool.tile([2 * U, 1], f32)
            nc.sync.dma_start(bzr_sb, bzr[:, :])
            b1h_sb = cpool.tile([U, 1], f32)
            nc.sync.dma_start(b1h_sb, b1h[:, :])
            b0h_sb = cpool.tile([U, 1], f32)
            nc.sync.dma_start(b0h_sb, b0h[:, :])

            # Route consts through DVE copies so PE/Act instrs wait on
            # compute semaphores, not raw DMA semaphores (LDW 1-wait limit).
            def dve_copy(src, shape, dt):
                dst = cpool.tile(shape, dt, name=src.tensor.name + "_c")
                nc.vector.tensor_copy(dst, src)
                return dst

            wzr_c = dve_copy(wzr_sb, [F, 2 * U], bf16)
            wh_c = dve_copy(wh_sb, [F, U], bf16)
            uzr_c = dve_copy(uzr_sb, [U, 2 * U], bf16)
            uh_c = dve_copy(uh_sb, [U, U], bf16)
            bzr_c = dve_copy(bzr_sb, [2 * U, 1], f32)
            b1h_c = dve_copy(b1h_sb, [U, 1], f32)
            b0h_c = dve_copy(b0h_sb, [U, 1], f32)

            # ---- recurrent state (ping-pong, bf16) ----
            hA = hpool.tile([U, BC], bf16, name="hA")
            hB = hpool.tile([U, BC], bf16, name="hB")
            nc.vector.memset(hA, 0.0)

            xs_tiles = {}

            def emit_dma(c):
                xsb = xpool.tile([F, TCH, BC], bf16, name="xsb")
                nc.sync.dma_start(xsb, xT[:, c * TCH:(c + 1) * TCH, :])
                xs_tiles[c] = xsb

            with (
                tc.tile_pool(name="pzr", bufs=2, space="PSUM") as pZR,
                tc.tile_pool(name="pxh", bufs=2, space="PSUM") as pXH,
                tc.tile_pool(name="ph", bufs=3, space="PSUM") as pH,
            ):
                # absorb DVE const-copy sem threshold on PE
                dummy = pH.tile([U, BC], f32, name="rhps")
                nc.tensor.matmul(dummy, uh_c, uh_c, start=True, stop=True)

                pzr_w = {}
                pxh_w = {}

                def emit_bulk(w):
                    c = (w * WS) // TCH
                    xsb = xs_tiles[c]
                    lt = w * WS - c * TCH
                    xsl = xsb[:, lt:lt + WS, :]
                    pzr = pZR.tile([2 * U, WS * BC], f32, name="pzr")
                    pxh = pXH.tile([U, WS * BC], f32, name="pxh")
                    nc.tensor.matmul(pzr, wzr_c, xsl, start=True, stop=False,
                                     skip_group_check=True)
                    nc.tensor.matmul(pxh, wh_c, xsl, start=True, stop=True,
                                     skip_group_check=True)
                    pzr_w[w] = pzr
                    pxh_w[w] = pxh

                emit_dma(0)
                emit_dma(1)
                emit_bulk(0)

                hcur = hA
                hnxt = hB
                D_prev = None
                xh_sb = {}
                for t in range(T):
                    w = t // WS
                    sl = slice((t % WS) * BC, (t % WS + 1) * BC)
                    pzr = pzr_w[w]
                    pxh = pxh_w[w]

                    if t >= 1:
                        # critical: close pzr[t] with the D-delta accumulate
                        nc.tensor.matmul(pzr[:, sl], uzr_c, D_prev,
                                         start=False, stop=True,
                                         skip_group_check=True)
                    S = spool.tile([2 * U, BC], bf16, name="S")
                    if nzzr:
                        nc.scalar.activation(S, pzr[:, sl], AF.Sigmoid,
                                             bias=bzr_c, scale=1.0)
                    else:
                        nc.scalar.activation(S, pzr[:, sl], AF.Sigmoid)
                    # stage xh for step t+1 to SBUF (off critical path)
                    if t + 1 < T:
                        nw_, nt_ = (t + 1) // WS, (t + 1) % WS
                        xh_n = xhpool.tile([U, BC], bf16, name="xh")
                        nc.scalar.copy(
                            xh_n, pxh_w[nw_][:, nt_ * BC:(nt_ + 1) * BC])
                        xh_sb[t + 1] = xh_n
                    if t >= 1:
                        ph = pH.tile([U, BC], f32, name="rhps")
                        nc.tensor.matmul(ph, uh_c, hcur, start=True, stop=True,
                                         skip_group_check=True)
                        if t + 1 < T:
                            # early accumulate of U^T h_t into pzr[t+1]
                            nxt_w = (t + 1) // WS
                            nsl = slice(((t + 1) % WS) * BC,
                                        ((t + 1) % WS + 1) * BC)
                            nc.tensor.matmul(pzr_w[nxt_w][:, nsl], uzr_c, hcur,
                                             start=False, stop=False,
                                             skip_group_check=True)
                    if t % WS == 0 and w + 1 < NW:
                        emit_bulk(w + 1)
                    if t % TCH == 0 and t // TCH + 2 < NCHUNK:
                        emit_dma(t // TCH + 2)

                    # ---- elementwise chain (DVE) ----
                    if t == 0:
                        C = dpool.tile([U, BC], bf16, name="C")
                        nc.vector.tensor_scalar_max(C, pxh[:, sl], 0.0)
                        D = ddpool.tile([U, BC], bf16, name="D")
                        nc.vector.tensor_mul(D, S[:U, :], C)
                        nc.vector.tensor_add(hnxt, hcur, D)
                    else:
                        A = dpool.tile([U, BC], bf16, name="A")
                        if nzrec:
                            nc.vector.scalar_tensor_tensor(
                                A, ph, b1h_c, S[U:2 * U, :],
                                op0=OP.add, op1=OP.mult)
                        else:
                            nc.vector.tensor_mul(A, S[U:2 * U, :], ph)
                        Bt = dpool.tile([U, BC], bf16, name="Bt")
                        if nzb0h:
                            nc.vector.scalar_tensor_tensor(
                                Bt, A, b0h_c, xh_sb[t],
                                op0=OP.add, op1=OP.add)
                        else:
                            nc.vector.tensor_add(Bt, A, xh_sb[t])
                        C = dpool.tile([U, BC], bf16, name="C")
                        nc.vector.scalar_tensor_tensor(
                            C, Bt, 0.0, hcur, op0=OP.max, op1=OP.subtract)
                        D = ddpool.tile([U, BC], bf16, name="D")
                        nc.vector.tensor_mul(D, S[:U, :], C)
                        nc.vector.tensor_add(hnxt, hcur, D)

                    D_prev = D
                    hcur, hnxt = hnxt, hcur

            nc.sync.dma_start(hout[:, :], hcur)

    nc.finalize()
    return nc


_CACHE = {}


def kernel(**inputs) -> np.ndarray:
    global LAST_RESULTS
    x = np.asarray(inputs["inputs"], dtype=np.float32)
    W = np.asarray(inputs["W"], dtype=np.float32)
    Um = np.asarray(inputs["U"], dtype=np.float32)
    b = np.asarray(inputs["b"], dtype=np.float32)
    W1 = np.asarray(inputs["W1"], dtype=np.float32)
    b1 = np.asarray(inputs["b1"], dtype=np.float32)
    W2 = np.asarray(inputs["W2"], dtype=np.float32)
    b2 = np.asarray(inputs["b2"], dtype=np.float32)

    nzrec = bool(np.any(b[1, 2 * U:]))
    nzb0h = bool(np.any(b[0, 2 * U:]))
    bsum = b[0] + b[1]
    bzr = np.concatenate([-bsum[:U], bsum[U:2 * U]]).reshape(2 * U, 1)
    nzzr = bool(np.any(bzr))
    key = (nzrec, nzb0h, nzzr)
    if key not in _CACHE:
        _CACHE[key] = build_nc(nzrec, nzb0h, nzzr)
    nc = _CACHE[key]

    # negate z-columns so sigmoid(pre_z) directly yields 1-z
    Wmod = W.copy()
    Wmod[:, :U] *= -1.0
    Umod = Um.copy()
    Umod[:, :U] *= -1.0
    common = {
        "Wzr": np.ascontiguousarray(Wmod[:, :2 * U]).astype(BF),
        "Wh": np.ascontiguousarray(Wmod[:, 2 * U:]).astype(BF),
        "Uzr": np.ascontiguousarray(Umod[:, :2 * U]).astype(BF),
        "Uh": np.ascontiguousarray(Umod[:, 2 * U:]).astype(BF),
        "bzr": np.ascontiguousarray(bzr, dtype=np.float32),
        "b1h": np.ascontiguousarray(b[1, 2 * U:].reshape(U, 1)),
        "b0h": np.ascontiguousarray(b[0, 2 * U:].reshape(U, 1)),
    }
    in_maps = []
    for c in range(NCORES):
        xc = np.ascontiguousarray(
            x[c * BC:(c + 1) * BC].transpose(2, 1, 0)).astype(BF)
        in_maps.append(dict(common, xT=xc))
    res = run_bass_kernel_spmd(nc, in_maps, core_ids=list(range(NCORES)),
                               trace=TRACE)
    LAST_RESULTS = res

    # MLP head + softmax on host (fp32)
    h_full = np.concatenate(
        [np.asarray(res.results[c]["hout"]).astype(np.float32).T
         for c in range(NCORES)], axis=0)            # [B, U]
    x1 = np.maximum(h_full @ W1 + b1, 0.0)
    lg = x1 @ W2 + b2
    e = np.exp(lg - lg.max(axis=-1, keepdims=True))
    return (e / e.sum(axis=-1, keepdims=True)).astype(np.float32)
